# revision 4
# baseline (speedup 1.0000x reference)
"""Quantized 3x3 conv (8-bit symmetric STE quantization of x and w, then
stride-1 pad-1 conv) on 8 Trainium2 NeuronCores.

Strategy (data-parallel over batch, 4 images per core):
  * x quantized on-device to integers in [-127,127] stored as bf16 via the
    fp32 magic-number round trick (3 elementwise passes, chunked by rows):
      P0: t = min(x*s, 127.25)          P1: v = max(t, -127.25) + 1.5*2^23
      P2: k = v - 1.5*2^23 -> bf16 into a zero-padded 58x58 grid
    P0/P1 on DVE (g0/g1 chunks interleaved in gating order -- DVE has a
    ~6us DVFS ramp from its first compute op and slows again when idle, so
    the queue is kept dense), P2 relayout on ACT.
  * conv = 9 shifted matmuls (K=ci=64) accumulating in PSUM; images 2g /
    2g+1 run concurrently on the PE's two 64-row tiles (partition halves).
  * taps-outer over groups of <=2 output-row blocks (8 rows x 58 cols = 464
    psum cols per block, bank-aligned); PSUM pool of 4x[128,1024] double-
    buffers groups.  Group patterns [1,2,2,2] (early start) / [2,2,2,1]
    (small final output DMA).
  * N_WARM warmup matmuls reading a memset tile start right after the
    framework preamble -- no DMA dependency -- so the PE's DVFS ramp
    (0.65/1.2/2.4 GHz p-states, ~12-14us to full clock measured from the
    first matmul) starts as early as possible and real matmuls spend less
    time at reduced clock.  The warm PSUM tile is read once into an SBUF
    sink that DMAs to a tiny dedicated DRAM output: leaving it unread
    wedged the device (pool rotation reused the bank mid-pipeline), and
    the old baseline's dummy-copy-into-os_[0] guard raced real output
    writes (~1/3 runs failed with rel err ~1).
  * outputs scaled+cast to bf16 on DVE (h=0) / ACT (h=1); both images of a
    pair go out in ONE dma_start per group via a [p, img, cols] view (DMA
    issues cost ~0.7us each on the issuing engine queue).  bf16 halves
    output traffic; measured end-to-end rel err ~3.4e-3 vs the fp32
    reference (harness gate 2e-2).
  * x chunks issue from SP in group-gating order, wq from ACT (both HWDGE).

Measured 47.3-48.1us HW exec (baseline: 49.3us warm / 58.5us cold), with
~6.5us framework preamble and a fixed ~6.5us end-of-kernel semaphore-sweep
epilogue inside the measured window.
"""

import numpy as np
import ml_dtypes

import concourse.bass as bass
import concourse.mybir as mybir
import concourse.tile as tile
from concourse import bacc
from concourse.bass_utils import run_bass_kernel_spmd

dt = mybir.dt

N_CORES = 8
NPC = 4                # images per core
CI, CO = 64, 128
H = W = 56
WP = 58                # padded row width (56 + 2)
LEAD = 4               # guard elems before the padded grid
IMG_ELEMS = LEAD + WP * WP + 8   # 3376
PACK = H * W           # 3136
MAGIC = 12582912.0     # 1.5 * 2^23 : fp32 round-to-nearest-even trick
CLIP = 127.25
H0S = [1 + 8 * i for i in range(7)]   # padded-row start of each 8-row block
BLK = 8 * WP           # 464 psum columns per block
N_WARM = 40            # 128-col warmups bridge the head until quant c0 lands

# row chunks for DMA + quant (data-row ranges); chunk i gates group i
QCH = [(0, 9), (9, 25), (25, 41), (41, 56)]
GROUPS = [
    [[0], [1, 2], [3, 4], [5, 6]],   # image pair 0: 1-block first group
    [[0, 1], [2, 3], [4, 5], [6]],   # image pair 1: 1-block last group
]

_PROG_CACHE = {}


def _build_program(s_x, s2):
    s_x = float(np.float32(s_x))
    s2 = float(np.float32(s2))
    nc = bacc.Bacc(None)
    x_in = nc.declare_dram_parameter("x", [NPC * CI, PACK], dt.float32, isOutput=False)
    wq_in = nc.declare_dram_parameter("wq", [128, 9, CO], dt.bfloat16, isOutput=False)
    out = nc.declare_dram_parameter("out", [NPC * CO, PACK], dt.bfloat16, isOutput=True)
    wsink = nc.declare_dram_parameter("wsink", [1, 8], dt.float32, isOutput=True)

    with tile.TileContext(nc) as tc:
        with (
            tc.tile_pool(name="sb", bufs=1) as sb,
            tc.tile_pool(name="ps", bufs=4, space="PSUM") as psp,
        ):
            wz = sb.tile([128, 512], dt.bfloat16)
            wq = sb.tile([128, 9, CO], dt.bfloat16)
            osink = sb.tile([1, 8], dt.float32)
            djunk = sb.tile([1, 256], dt.float32)
            xs = [sb.tile([128, PACK], dt.float32, name=f"xs{g}", tag=f"xs{g}")
                  for g in range(2)]
            x2 = [sb.tile([128, PACK], dt.float32, name=f"x2{g}", tag=f"x2{g}")
                  for g in range(2)]
            xq = [sb.tile([128, IMG_ELEMS], dt.bfloat16, name=f"xq{g}", tag=f"xq{g}")
                  for g in range(2)]
            # one tile per image pair: [p, (img, block, row, col)] so both
            # images' chunks go out in a single DMA per group
            os_ = [sb.tile([128, 2 * PACK], dt.bfloat16, name=f"os{g}", tag=f"os{g}")
                   for g in range(2)]
            outv = out.rearrange("(i p) x -> p i x", p=CO)

            # ---- PE warmup: starts as soon as the tiny memset lands ----
            # (DVFS p-state ramp is anchored at the first PE activity)
            nc.gpsimd.memset(wz[:], 0.0)
            warm = psp.tile([128, 1024], dt.float32, name="warm", tag="ps")
            for _ in range(N_WARM):
                nc.tensor.matmul(warm[:, 0:128], lhsT=wz[0:64, 0:128],
                                 rhs=wz[0:64, 0:128], start=True, stop=True)

            # DVE slow-start: its first 1-2 compute ops run ~6x slow no
            # matter the size; burn that warm-up on a dep-free dummy so the
            # real quant ops run at full rate (region is overwritten much
            # later by P1 chunk 3 -- WAW, no stall)
            nc.vector.memset(x2[0][:, 2880:3136], 0.0)
            nc.vector.tensor_scalar(
                out=x2[0][:, 2880:3136], in0=x2[0][:, 2880:3136],
                scalar1=1.0, scalar2=0.0,
                op0=mybir.AluOpType.mult, op1=mybir.AluOpType.add)

            # zero-pad grids (borders must be 0 for the shifted matmuls);
            # xq[0] first -- P2 of g0 c0 needs it ~3us before g1's
            nc.gpsimd.memset(xq[0][:], 0.0)
            nc.gpsimd.memset(xq[1][:], 0.0)

            # ---- input DMA: all x chunks on SP (g0/g1 interleaved in
            # gating order), wq on ACT ----
            # tiny dummy DMA first: absorbs the ~2us DMA-path arming latency
            # so the head-critical x chunk's data lands earlier
            nc.sync.dma_start(out=djunk[:], in_=x_in[0:1, 0:256])
            nc.scalar.dma_start(out=wq[:], in_=wq_in[:])
            # g0 chunk 0 is split (0,7)+(7,9): block-0's dh=0 taps need only
            # data rows 0-6, so the first real matmuls start ~1us earlier
            DMACH = [((0, 0), (0, 7)), ((0, 0), (7, 9))] + [
                ((g, ci), QCH[ci])
                for g, ci in [(0, 1), (1, 0), (0, 2), (1, 1),
                              (0, 3), (1, 2), (1, 3)]]
            for (g, _), (r0, r1) in DMACH:
                nc.sync.dma_start(
                    out=xs[g][:, r0 * W:r1 * W],
                    in_=x_in[128 * g:128 * (g + 1), r0 * W:r1 * W])

            # ---- quantization: P0/P1 on DVE (gpsimd tensor_scalar is
            # ~15ns/col software -- unusable); P2 relayout on ACT except
            # chunk 0 of g0 (DVE: no cross-engine hop on the head path).
            # Emission interleaves g0/g1 chunks in readiness order so the
            # DVE queue never idles long (sparse ops drop DVE to a slow
            # p-state). ----
            def quant_chunk(g, ci, rows=None):
                r0, r1 = rows if rows is not None else QCH[ci]
                x23 = x2[g].rearrange("p (r w) -> p r w", w=W)
                grid = xq[g][:, LEAD:LEAD + WP * WP].rearrange(
                    "p (r w) -> p r w", w=WP)
                cs = slice(r0 * W, r1 * W)
                nc.vector.tensor_scalar(
                    out=xs[g][:, cs], in0=xs[g][:, cs],
                    scalar1=s_x, scalar2=CLIP,
                    op0=mybir.AluOpType.mult, op1=mybir.AluOpType.min)
                nc.vector.tensor_scalar(
                    out=x2[g][:, cs], in0=xs[g][:, cs],
                    scalar1=-CLIP, scalar2=MAGIC,
                    op0=mybir.AluOpType.max, op1=mybir.AluOpType.add)
                nc.scalar.activation(
                    out=grid[:, 1 + r0:1 + r1, 1:57],
                    in_=x23[:, r0:r1, :],
                    func=mybir.ActivationFunctionType.Copy,
                    bias=-MAGIC, scale=1.0)

            quant_chunk(0, 0, rows=(0, 7))
            quant_chunk(0, 0, rows=(7, 9))
            for g, ci in [(0, 1), (1, 0), (0, 2), (1, 1),
                          (0, 3), (1, 2), (1, 3)]:
                quant_chunk(g, ci)
                if (g, ci) == (0, 1):
                    # read the warm tile so its PSUM slot is provably
                    # drained before the pool rotates it into a real
                    # accumulation tile (an unread warm tile wedged the
                    # device).  Here the DVE queue reaches it just after
                    # the warmups end -- no stall on either side.
                    nc.vector.tensor_copy(osink[:], warm[0:1, 0:8])

            # ---- conv: taps-outer per block group, dual PE tiles ----
            for g in range(2):
                for blocks in GROUPS[g]:
                    b0, nb = blocks[0], len(blocks)
                    ps_pair = [psp.tile([128, 1024], dt.float32,
                                        name=f"psum_g{g}b{b0}h{h}", tag="ps")
                               for h in range(2)]
                    ps2 = [p.rearrange("p (b x) -> p b x", b=2) for p in ps_pair]
                    for t in range(9):
                        dh, dw = t // 3, t % 3
                        for h in (1, 0):
                            for bi in range(nb):
                                off = (LEAD + (H0S[b0 + bi] + dh - 1) * WP
                                       + (dw - 1))
                                nc.tensor.matmul(
                                    ps2[h][:, bi, 0:BLK],
                                    lhsT=wq[64 * h:64 * (h + 1), t, :],
                                    rhs=xq[g][64 * h:64 * (h + 1), off:off + BLK],
                                    start=(t == 0), stop=(t == 8),
                                )
                    # scale, cast bf16, strip pad columns; one combined
                    # 2-image DMA per group (splitting the last group into
                    # row halves measured WORSE: two serialized ~0.64us DMA
                    # issues outweigh the smaller transfers)
                    osv = os_[g].rearrange("p (i b r w) -> p i b r w",
                                           i=2, r=8, w=W)
                    for h in range(2):
                        sel = ps2[h][:, 0:nb, 0:BLK].rearrange(
                            "p b (r w) -> p b r w", w=WP)[:, :, :, 1:57]
                        dst = osv[:, h, b0:b0 + nb]
                        if h == 0:
                            nc.vector.tensor_scalar_mul(
                                out=dst, in0=sel, scalar1=s2)
                        else:
                            nc.scalar.activation(
                                out=dst, in_=sel,
                                func=mybir.ActivationFunctionType.Copy,
                                scale=s2)
                    nc.sync.dma_start(
                        out=outv[:, 2 * g:2 * g + 2,
                                 448 * b0:448 * (b0 + nb)],
                        in_=os_[g].rearrange("p (i x) -> p i x", i=2)[
                            :, :, 448 * b0:448 * (b0 + nb)],
                    )
            nc.scalar.dma_start(out=wsink[:], in_=osink[:])
    if not nc.is_finalized():
        nc.finalize()
    return nc


def _host_prep(x, w, alpha_x, alpha_w):
    x = np.ascontiguousarray(np.asarray(x, dtype=np.float32))
    w = np.asarray(w, dtype=np.float32)
    ax = np.float32(max(np.float32(np.asarray(alpha_x).reshape(-1)[0]), np.float32(0)))
    aw = np.float32(max(np.float32(np.asarray(alpha_w).reshape(-1)[0]), np.float32(0)))
    step_x = np.float32(np.float32(np.float32(2.0) * ax) / np.float32(254.0))
    step_w = np.float32(np.float32(np.float32(2.0) * aw) / np.float32(254.0))
    s_x = np.float32(np.float32(1.0) / step_x)
    s2 = np.float32(step_x * step_w)

    kw = np.clip(np.round((w / step_w).astype(np.float32)), -127, 127)
    kw = kw.reshape(CO, CI, 9).transpose(1, 2, 0)          # [ci, tap, co]
    wq = np.concatenate([kw, kw], axis=0).astype(ml_dtypes.bfloat16)
    return x, wq, s_x, s2


def _in_maps(x, wq):
    return [
        {
            "x": x[NPC * c:NPC * (c + 1)].reshape(NPC * CI, PACK),
            "wq": wq,
        }
        for c in range(N_CORES)
    ]


def get_program(s_x=127.0, s2=float(np.float32(np.float32(1 / np.float32(127.0)) ** 2))):
    key = (float(np.float32(s_x)), float(np.float32(s2)))
    if key not in _PROG_CACHE:
        _PROG_CACHE[key] = _build_program(*key)
    return _PROG_CACHE[key]


def run_on_hw(x, w, alpha_x, alpha_w, trace=False):
    xx, wq, s_x, s2 = _host_prep(x, w, alpha_x, alpha_w)
    nc = get_program(s_x, s2)
    res = run_bass_kernel_spmd(nc, _in_maps(xx, wq),
                               list(range(N_CORES)), trace=trace)
    out = np.concatenate(
        [np.asarray(res.results[i]["out"]).astype(np.float32).reshape(NPC, CO, H, W)
         for i in range(N_CORES)], axis=0)
    return out, res


def kernel(x, w, alpha_x, alpha_w):
    out, _ = run_on_hw(x, w, alpha_x, alpha_w)
    return out
